# revision 1
# baseline (speedup 1.0000x reference)
"""Distributed Bass kernel for nn_Attention_65025804861926 on 8 TRN2 NeuronCores.

Reference computation (B=4, S=8192, D=1024):
    xq = LN(x @ wq.T) ; xk = LN(x @ wk.T) ; xv = x @ wv.T        [B,S,D]
    scores = einsum('bsi,bsj->bij', xq, xk)                       [B,D,D]
    attn = softmax(scores, -1)
    out = einsum('bij,bsj->bsi', attn, xv) @ wo.T                 [B,S,D]

Sharding: the 4x8192 (b,s) rows are split over 8 cores (4096 rows each,
two cores per batch).  The D x D score matrix needs the sum over the full
sequence, so the two cores of a pair ReduceScatter their partial scores
(each keeps 512 of the 1024 softmax rows), softmax locally, and AllGather
the transposed attention halves.  Weights are replicated.

All matmuls run in fp16 (fp32 PSUM accumulation); empirically this gives
~5e-3 relative error end-to-end vs the fp32 reference (the softmax is
near-one-hot, so the Q/K path needs fp16's 11 mantissa bits; bf16 fails).
"""

import sys

for _p in ("/opt/trn_rl_repo",):
    if _p not in sys.path:
        sys.path.append(_p)

import numpy as np

import concourse.bass as bass
import concourse.tile as tile
from concourse import bacc, mybir
from concourse.bass_utils import run_bass_kernel_spmd
from concourse.masks import make_identity

P = 128
D = 1024
FC = D // P            # 8 feature chunks of 128
NC_HALF = 512          # matmul moving-dim / PSUM free size
F32 = mybir.dt.float32
F16 = mybir.dt.float16
AX = mybir.AxisListType
ALU = mybir.AluOpType
ACTF = mybir.ActivationFunctionType

GROUPS = [[0, 1], [2, 3], [4, 5], [6, 7]]
EPS = 1e-5


def _load_weight_half(nc, pools, w_ext, name, wT, h):
    """Stage rows [h*512, (h+1)*512) of a [D, D] fp32 weight (fp16 casting
    DMA) and TensorE-transpose them into wT[:, :, h*512:(h+1)*512].
    Four transpose blocks share one PSUM tile so PSUM->SBUF copies move
    [128, 512] at a time."""
    stage_pool, ps_pool, ident16 = pools
    half = FC // 2
    w16 = stage_pool.tile([P, half, D], F16, tag="wstage", name=f"{name}_nat{h}", bufs=2)
    nc.gpsimd.dma_start(
        out=w16[:],
        in_=w_ext[h * half * P:(h + 1) * half * P, :].rearrange(
            "(io p) f -> p io f", p=P))
    for fo in range(FC):
        ps = ps_pool.tile([P, 4 * P], F16, tag="tps", name=f"{name}_ps")
        for q in range(4):
            nc.tensor.transpose(ps[:, q * P:(q + 1) * P],
                                w16[:, q, fo * P:(fo + 1) * P], ident16[:])
        nc.scalar.copy(out=wT[:, fo, h * 4 * P:(h + 1) * 4 * P], in_=ps[:])


def _load_weight_transposed(nc, pools, w_ext, name):
    wpool, stage_pool, ps_pool, ident16 = pools
    wT = wpool.tile([P, FC, D], F16, name=f"{name}T")
    for h in range(2):
        _load_weight_half(nc, (stage_pool, ps_pool, ident16), w_ext, name, wT, h)
    return wT


def build_attention_nc(rows=4096, sb_tiles=8, g_tiles=4, xv_bufs=3, collectives=True):
    """Build the SPMD graph (identical on all 8 cores)."""
    NT = rows // P                       # row tiles per core
    NSB = NT // sb_tiles                 # scores superblocks
    NG = NT // g_tiles                   # pass-2 groups
    GS = g_tiles * P                     # rows per pass-2 group
    IO_HALF = D // 2 // P                # softmax row chunks (4)

    nc = bacc.Bacc(None, num_devices=8)

    x_ext = nc.dram_tensor("x", [rows, D], F32, kind="ExternalInput")
    w_ext = {w: nc.dram_tensor(w, [D, D], F32, kind="ExternalInput")
             for w in ("wq", "wk", "wv", "wo")}
    gb_ext = {g: nc.dram_tensor(g, [D], F32, kind="ExternalInput")
              for g in ("q_gamma", "q_beta", "k_gamma", "k_beta")}
    out_ext = nc.dram_tensor("out", [rows, D], F32, kind="ExternalOutput")

    x_view = x_ext[:].rearrange("(n p) d -> n p d", p=P)      # [NT, 128, D]
    out_view = out_ext[:].rearrange("(n p) d -> n p d", p=P)

    with tile.TileContext(nc) as tc:
        from contextlib import ExitStack

        with ExitStack() as persist:
            wpool = persist.enter_context(tc.tile_pool(name="weights", bufs=1))
            cpool = persist.enter_context(tc.tile_pool(name="consts", bufs=1))
            dram = persist.enter_context(tc.tile_pool(name="dram", bufs=1, space="DRAM"))

            ident16 = cpool.tile([P, P], F16)
            make_identity(nc, ident16)

            eps_sb = cpool.tile([P, 1], F32)
            nc.vector.memset(eps_sb[:], EPS)

            def load_gamma_beta():
                # deferred: these SWDGE loads must queue behind the weight
                # staging (they're only needed at the first layernorm)
                out = {}
                for g in ("q_gamma", "q_beta", "k_gamma", "k_beta"):
                    t = cpool.tile([P, D], F32, name=f"{g}_sb")
                    src = gb_ext[g][:]
                    bcast = bass.AP(tensor=src.tensor, offset=src.offset,
                                    ap=[[0, P]] + list(src.ap))
                    nc.gpsimd.dma_start(out=t[:], in_=bcast)
                    out[g] = t
                return out

            # ---------------- pass 1: Q/K projections + LN + scores ----------
            with ExitStack() as p1:
                qkw = p1.enter_context(tc.tile_pool(name="qkw", bufs=1))
                stage = p1.enter_context(tc.tile_pool(name="wstage", bufs=1))
                ps_t = p1.enter_context(tc.tile_pool(name="ps_t", bufs=2, space="PSUM"))
                psA = p1.enter_context(tc.tile_pool(name="psA", bufs=6, space="PSUM"))
                p1pool = p1.enter_context(tc.tile_pool(name="p1", bufs=2))
                sbq = p1.enter_context(tc.tile_pool(name="sbq", bufs=1))
                accp = p1.enter_context(tc.tile_pool(name="accp", bufs=1))

                _sid_p1, _ = nc.enter_named_scope("p1", False)

                # interleave the q/k weight halves so the first projection
                # matmuls (which only need the h=0 columns) start early;
                # the first x tiles slot into the SWDGE queue between halves
                wqT = qkw.tile([P, FC, D], F16, name="wqT")
                wkT = qkw.tile([P, FC, D], F16, name="wkT")
                _load_weight_half(nc, (stage, ps_t, ident16), w_ext["wq"], "wq", wqT, 0)
                x_pre = {}
                for gt in range(min(3, NT)):
                    t = p1pool.tile([P, D], F16, tag="x16", name="x16", bufs=3)
                    nc.gpsimd.dma_start(out=t[:], in_=x_view[gt])
                    x_pre[gt] = t
                _load_weight_half(nc, (stage, ps_t, ident16), w_ext["wk"], "wk", wkT, 0)
                _load_weight_half(nc, (stage, ps_t, ident16), w_ext["wq"], "wq", wqT, 1)
                _load_weight_half(nc, (stage, ps_t, ident16), w_ext["wk"], "wk", wkT, 1)
                gb_sb = load_gamma_beta()
                wvT = None

                scores_acc = accp.tile([P, FC, D], F32)   # [i%P, i//P, j]
                xT_dram = dram.tile([P, FC, NT * P], F16)  # transposed-x cache for pass 2
                scores_dram = dram.tile([D, D], F32)

                def stage_tile(gt):
                    """x load + TensorE transpose + xT cache write for one tile."""
                    if gt in x_pre:
                        x16 = x_pre.pop(gt)
                    else:
                        x16 = p1pool.tile([P, D], F16, tag="x16", name="x16", bufs=3)
                        nc.gpsimd.dma_start(out=x16[:], in_=x_view[gt])
                    xT16 = p1pool.tile([P, FC, P], F16, tag="xT16", name="xT16", bufs=3)
                    for fq in range(2):
                        ps = ps_t.tile([P, 4 * P], F16, tag="tps", name="xt_ps")
                        for q in range(4):
                            fc = fq * 4 + q
                            nc.tensor.transpose(ps[:, q * P:(q + 1) * P],
                                                x16[:, fc * P:(fc + 1) * P], ident16[:])
                        nc.scalar.copy(out=xT16[:, fq * 4:(fq + 1) * 4, :], in_=ps[:])
                    nc.sync.dma_start(out=xT_dram[:, :, gt * P:(gt + 1) * P], in_=xT16[:])
                    return xT16

                xT_staged = {0: stage_tile(0)}

                for sb in range(NSB):
                    xq16 = sbq.tile([P, sb_tiles, D], F16, tag="xq16", name="xq16")
                    xk16 = sbq.tile([P, sb_tiles, D], F16, tag="xk16", name="xk16")

                    for t in range(sb_tiles):
                        gt = sb * sb_tiles + t
                        # transpose the NEXT tile first: its PSUM->SBUF copies
                        # then hide under this tile's projection matmuls
                        if gt + 1 < NT and gt + 1 not in xT_staged:
                            xT_staged[gt + 1] = stage_tile(gt + 1)
                        xT16 = xT_staged.pop(gt)

                        q_ps = [psA.tile([P, NC_HALF], F32, tag="mm", name="q_ps") for _ in range(2)]
                        k_ps = [psA.tile([P, NC_HALF], F32, tag="mm", name="k_ps") for _ in range(2)]
                        for h in range(2):
                            sl = slice(h * NC_HALF, (h + 1) * NC_HALF)
                            for tgt, wT in ((q_ps[h], wqT), (k_ps[h], wkT)):
                                for fc in range(FC):
                                    nc.tensor.matmul(tgt[:], xT16[:, fc, :], wT[:, fc, sl],
                                                     start=(fc == 0), stop=(fc == FC - 1))

                        # layernorm  (q - mu) * rstd * gamma + beta  -> fp16
                        for which, w_ps, dst in (("q", q_ps, xq16), ("k", k_ps, xk16)):
                            gam = gb_sb[f"{which}_gamma"]
                            bet = gb_sb[f"{which}_beta"]
                            stats = p1pool.tile([P, 2, 6], F32, tag="stats", name="stats", bufs=4)
                            nc.vector.bn_stats(out=stats[:, 0, :], in_=w_ps[0][:])
                            nc.vector.bn_stats(out=stats[:, 1, :], in_=w_ps[1][:])
                            mv = p1pool.tile([P, 2], F32, tag="mv", name="mv", bufs=4)
                            nc.vector.bn_aggr(out=mv[:], in_=stats[:])
                            tmp = p1pool.tile([P, D], F32, tag="lntmp", name="lntmp", bufs=2)
                            # read the PSUM halves first so the projection PSUM
                            # frees before the DVE waits on the ACT sqrt
                            for h in range(2):
                                sl = slice(h * NC_HALF, (h + 1) * NC_HALF)
                                nc.vector.scalar_tensor_tensor(
                                    out=tmp[:, sl], in0=w_ps[h][:], scalar=mv[:, 0:1],
                                    in1=gam[:, sl], op0=ALU.subtract, op1=ALU.mult)
                            rstd = p1pool.tile([P, 1], F32, tag="rstd", name="rstd", bufs=4)
                            nc.scalar.activation(out=rstd[:], in_=mv[:, 1:2], func=ACTF.Sqrt,
                                                 bias=eps_sb[:], scale=1.0)
                            nc.vector.reciprocal(out=rstd[:], in_=rstd[:])
                            for h in range(2):
                                sl = slice(h * NC_HALF, (h + 1) * NC_HALF)
                                nc.vector.scalar_tensor_tensor(
                                    out=dst[:, t, sl], in0=tmp[:, sl], scalar=rstd[:],
                                    in1=bet[:, sl], op0=ALU.mult, op1=ALU.add)

                    # scores partial accumulation for this superblock
                    for ic in range(FC):
                        for jc in range(2):
                            sc_ps = psA.tile([P, NC_HALF], F32, tag="mm", name="sc_ps")
                            for t in range(sb_tiles):
                                nc.tensor.matmul(
                                    sc_ps[:],
                                    xq16[:, t, ic * P:(ic + 1) * P],
                                    xk16[:, t, jc * NC_HALF:(jc + 1) * NC_HALF],
                                    start=(t == 0), stop=(t == sb_tiles - 1))
                            dst = scores_acc[:, ic, jc * NC_HALF:(jc + 1) * NC_HALF]
                            if sb == 0:
                                nc.vector.tensor_copy(dst, sc_ps[:])
                            else:
                                nc.vector.tensor_add(out=dst, in0=dst, in1=sc_ps[:])
                        if sb == NSB - 1:
                            # final value for this ic row block: ship it now
                            nc.sync.dma_start(out=scores_dram[ic * P:(ic + 1) * P, :],
                                              in_=scores_acc[:, ic, :])

                    if sb == 0:
                        # stage wv behind superblock 0 so its DMA doesn't
                        # delay the first x tiles; PE transposes slot in here
                        wvT = _load_weight_transposed(nc, (wpool, stage, ps_t, ident16), w_ext["wv"], "wv")
                        # prefetch the first V group's transposed-x while the
                        # pass-1 pools still own the rest of SBUF
                        xTg0 = cpool.tile([P, FC, GS], F16, name="xTg0")
                        nc.sync.dma_start(out=xTg0[:], in_=xT_dram[:, :, 0:GS])

                nc.leave_named_scope("p1", _sid_p1, False)
                _sid_rs, _ = nc.enter_named_scope("rs", False)
                rs_out = dram.tile([D // 2, D], F32)
                if collectives:
                    nc.gpsimd.collective_compute(
                        "ReduceScatter", ALU.add, replica_groups=GROUPS,
                        ins=[scores_dram.opt()], outs=[rs_out.opt()])
                else:
                    nc.sync.dma_start(out=rs_out[:], in_=scores_dram[0:D // 2])
                nc.leave_named_scope("rs", _sid_rs, False)

            # ---------------- pass 2: V, softmax, attention, output ----------
            with ExitStack() as p2:
                ps_t2 = p2.enter_context(tc.tile_pool(name="ps_t2", bufs=2, space="PSUM"))
                psB = p2.enter_context(tc.tile_pool(name="psB", bufs=6, space="PSUM"))
                p2pool = p2.enter_context(tc.tile_pool(name="p2", bufs=2))
                vpool = p2.enter_context(tc.tile_pool(name="vpool", bufs=xv_bufs))
                smpool = p2.enter_context(tc.tile_pool(name="smpool", bufs=1))
                stage2 = p2.enter_context(tc.tile_pool(name="wstage2", bufs=1))

                xv_dram = dram.tile([NG, P, FC * GS], F16)

                def v_group_start(g):
                    if g == 0:
                        xTg = xTg0
                    else:
                        xTg = p2pool.tile([P, FC, GS], F16, tag="xTg", name="xTg")
                        nc.sync.dma_start(out=xTg[:], in_=xT_dram[:, :, g * GS:(g + 1) * GS])
                    xv_g = vpool.tile([P, FC, GS], F16, tag="xv", name="xv_g")
                    return xTg, xv_g

                def v_jc(xTg, xv_g, jc):
                    v_ps = psB.tile([P, GS], F32, tag="mm2", name="v_ps")
                    for fc in range(FC):
                        nc.tensor.matmul(v_ps[:], wvT[:, fc, jc * P:(jc + 1) * P],
                                         xTg[:, fc, :],
                                         start=(fc == 0), stop=(fc == FC - 1))
                    nc.vector.tensor_copy(xv_g[:, jc, :], v_ps[:])

                def v_group_finish(g, xv_g):
                    nc.sync.dma_start(out=xv_dram[g],
                                      in_=xv_g[:].rearrange("p a b -> p (a b)"))

                _sid_v, _ = nc.enter_named_scope("vproj", False)
                # V projection for groups 0..NG-2 (overlaps the ReduceScatter);
                # the last group interleaves with the softmax transposes below
                for g in range(NG - 1):
                    xTg, xv_g = v_group_start(g)
                    for jc in range(FC):
                        v_jc(xTg, xv_g, jc)
                    v_group_finish(g, xv_g)
                nc.leave_named_scope("vproj", _sid_v, False)

                _sid_sm, _ = nc.enter_named_scope("softmax_ag", False)
                # softmax over own 512 rows: DVE/ACT chains first (no PE),
                # then PE alternates last-V-group matmuls with the transposes
                rs_view = rs_out[:].rearrange("(io p) j -> p io j", p=P)
                attn_tiles = []
                for io in range(IO_HALF):
                    sm = p2pool.tile([P, D], F32, tag="smio", name="sm", bufs=2)
                    nc.sync.dma_start(out=sm[:], in_=rs_view[:, io, :])
                    negmax = p2pool.tile([P, 1], F32, tag="negmax", name="negmax", bufs=4)
                    nc.vector.reduce_max(out=negmax[:], in_=sm[:], axis=AX.X, negate=True)
                    sumexp = p2pool.tile([P, 1], F32, tag="sumexp", name="sumexp", bufs=4)
                    nc.scalar.activation(out=sm[:], in_=sm[:], func=ACTF.Exp,
                                         bias=negmax[:], scale=1.0, accum_out=sumexp[:])
                    rsum = p2pool.tile([P, 1], F32, tag="rsum", name="rsum", bufs=4)
                    nc.vector.reciprocal(out=rsum[:], in_=sumexp[:])
                    attn16 = p2pool.tile([P, D], F16, tag="attn16", name="attn16", bufs=4)
                    nc.vector.tensor_scalar_mul(attn16[:], sm[:], rsum[:])
                    attn_tiles.append(attn16)

                agin = smpool.tile([P, FC, D // 2], F16)
                xTg7, xv_g7 = v_group_start(NG - 1)
                for io in range(IO_HALF):
                    # two V matmul groups keep the PE fed while softmax chunk
                    # `io` finishes on DVE/ACT
                    v_jc(xTg7, xv_g7, 2 * io)
                    v_jc(xTg7, xv_g7, 2 * io + 1)
                    attn16 = attn_tiles[io]
                    for jq in range(2):
                        ps = ps_t2.tile([P, 4 * P], F16, tag="tps", name="at_ps")
                        for q in range(4):
                            jc = jq * 4 + q
                            nc.tensor.transpose(ps[:, q * P:(q + 1) * P],
                                                attn16[:, jc * P:(jc + 1) * P], ident16[:])
                        nc.scalar.copy(out=agin[:, jq * 4:(jq + 1) * 4, io * P:(io + 1) * P],
                                       in_=ps[:].rearrange("p (q c) -> p q c", q=4))
                v_group_finish(NG - 1, xv_g7)

                agin_dram = dram.tile([D, D // 2], F16)
                nc.sync.dma_start(out=agin_dram[:].rearrange("(jc p) i -> p jc i", p=P), in_=agin[:])
                agout_dram = dram.tile([2 * D, D // 2], F16)
                if collectives:
                    nc.gpsimd.collective_compute(
                        "AllGather", ALU.bypass, replica_groups=GROUPS,
                        ins=[agin_dram.opt()], outs=[agout_dram.opt()])
                else:
                    nc.sync.dma_start(out=agout_dram[0:D], in_=agin_dram[:])
                    nc.sync.dma_start(out=agout_dram[D:2 * D], in_=agin_dram[:])

                # wo prep lands here: PE work while the AllGather is in flight
                woT = _load_weight_transposed(nc, (wpool, stage2, ps_t2, ident16), w_ext["wo"], "wo")

                attnT = smpool.tile([P, FC, D], F16)
                nc.sync.dma_start(out=attnT[:, :, 0:D // 2],
                                  in_=agout_dram[0:D].rearrange("(jc p) i -> p jc i", p=P))
                nc.sync.dma_start(out=attnT[:, :, D // 2:D],
                                  in_=agout_dram[D:2 * D].rearrange("(jc p) i -> p jc i", p=P))
                nc.leave_named_scope("softmax_ag", _sid_sm, False)

                _sid_ab, _ = nc.enter_named_scope("attn_out", False)
                for g in range(NG):
                    xv_g = vpool.tile([P, FC, GS], F16, tag="xv2", name="xv_g2", bufs=2)
                    nc.sync.dma_start(out=xv_g[:].rearrange("p a b -> p (a b)"),
                                      in_=xv_dram[g])
                    outT = p2pool.tile([P, FC, GS], F16, tag="outT", name="outT")
                    for ic in range(FC):
                        o_ps = psB.tile([P, GS], F32, tag="mm2", name="o_ps")
                        for jc in range(FC):
                            nc.tensor.matmul(o_ps[:], attnT[:, jc, ic * P:(ic + 1) * P],
                                             xv_g[:, jc, :],
                                             start=(jc == 0), stop=(jc == FC - 1))
                        nc.vector.tensor_copy(outT[:, ic, :], o_ps[:])
                    for ss in range(g_tiles):
                        f_ps = [psB.tile([P, NC_HALF], F32, tag="mm2", name="f_ps") for _ in range(2)]
                        for ic in range(FC):
                            lhs = outT[:, ic, ss * P:(ss + 1) * P]
                            st = dict(start=(ic == 0), stop=(ic == FC - 1))
                            for h in range(2):
                                nc.tensor.matmul(f_ps[h][:], lhs,
                                                 woT[:, ic, h * NC_HALF:(h + 1) * NC_HALF], **st)
                        out_sb = p2pool.tile([P, D], F32, tag="out_sb", name="out_sb", bufs=2)
                        for h in range(2):
                            nc.scalar.copy(out=out_sb[:, h * NC_HALF:(h + 1) * NC_HALF], in_=f_ps[h][:])
                        nc.sync.dma_start(out=out_view[g * g_tiles + ss], in_=out_sb[:])

                nc.leave_named_scope("attn_out", _sid_ab, False)

    nc.compile()
    return nc


_NC_CACHE = {}


def _get_nc(rows=4096):
    if rows not in _NC_CACHE:
        _NC_CACHE[rows] = build_attention_nc(rows=rows)
    return _NC_CACHE[rows]


def _shard_inputs(inputs, rows=4096):
    x = np.ascontiguousarray(np.asarray(inputs["x"], dtype=np.float32))
    B, S, Dd = x.shape
    per = {}
    for k in ("wq", "wk", "wv", "wo", "q_gamma", "q_beta", "k_gamma", "k_beta"):
        per[k] = np.ascontiguousarray(np.asarray(inputs[k], dtype=np.float32))
    halves = S // rows
    in_maps = []
    for c in range(8):
        b, h = c // halves, c % halves
        m = {"x": np.ascontiguousarray(x[b, h * rows:(h + 1) * rows, :])}
        m.update(per)
        in_maps.append(m)
    return in_maps


def run(inputs, trace=False, **kwargs):
    rows = 4096
    nc = _get_nc(rows)
    in_maps = _shard_inputs(inputs, rows)
    res = run_bass_kernel_spmd(nc, in_maps, core_ids=list(range(8)), trace=trace, **kwargs)
    x = np.asarray(inputs["x"])
    B, S, Dd = x.shape
    halves = S // rows
    out = np.empty((B, S, Dd), dtype=np.float32)
    for c in range(8):
        b, h = c // halves, c % halves
        out[b, h * rows:(h + 1) * rows, :] = res.results[c]["out"]
    return out, res


def kernel(**inputs):
    out, _ = run(inputs, trace=False)
    return out


if __name__ == "__main__":
    nc = build_attention_nc(rows=512, sb_tiles=2, g_tiles=2, xv_bufs=2)
    print("built ok:", len([i for bb in nc.main_func.blocks for i in bb.instructions]), "instructions")

